# revision 41
# baseline (speedup 1.0000x reference)
"""Trainium2 Bass kernel for nn_LocalizedFiltering (fused cat-conv2d x2 + residual + RMSNorm).

Strategy: sequence-parallel across 8 NeuronCores (one sequence of 2048 tokens +
1 cache row per core) -- no collectives needed.

Layer 1 runs feature-on-partition (output o1T = [1024 feat, 2049 tok] bf16 in
SBUF); layer 2 runs token-on-partition (lhsT = o1T column windows, rhs = W2
rows), so its PSUM output is already row-major [128 tok, 512 feat] -- no PE
transposes, and the residual + RMSNorm epilogue works directly on token rows.
The kernel-2 causal conv's shift-add is absorbed as two accumulated matmul
windows in both layers. conv2_bias is folded into the residual on the host
(xrb2 = x + b2); ln_weight is applied exactly on the host.

Matmuls run in bf16 (fp32 PSUM accumulation), except the first FP8U layer-1
k-tiles which run as fp8e4m3 DoubleRow "split" matmuls: the two DoubleRow
slots hold (x_hi, x_lo-residual) of the SAME k-tile against (W*S8, W*S8/C8),
so the x-quantization error cancels to second order and only the W-error
remains (~1.2% per tile, accumulating as sqrt(FP8U)).  Scales S8/C8 keep the
small operands out of e4m3's denormal range; slot products compensate
exactly.  Measured 1.82e-2 absmax/scale vs the 2e-2 gate.  Epilogue in fp32.
"""

import os

import numpy as np
import ml_dtypes

BS, L, D, CACHE = 8, 2048, 2048, 64
T = BS * L
H = D // 2          # 1024
EPS = 1e-6
NCORES = 8
BLK = 512           # token block (= one PSUM bank of fp32)
NBLK = L // BLK     # 4
KT1 = D // 128      # 16 contraction tiles, layer 1
KT2 = H // 128      # 8 contraction tiles, layer 2
QT1 = H // 128      # 8 output-feature tiles, layer 1 (per half)
TT = L // 128       # 16 token tiles, layer 2
FP8U = 13           # leading layer-1 k-tiles as fp8 DoubleRow "split" units:
                    # slot0 = q(x/S8)@q(W*S8), slot1 = q(C8*(x/S8-slot0))@q(W*S8/C8)
                    # -- the x-quantization error is second order, only the
                    # W-error remains, so error grows ~sqrt(FP8U)*1.2%/tile.
S8 = 32.0           # hi-slot scale balance (keeps W*S8 out of e4m3 denormals)
C8 = 32.0           # lo-slot rescale (keeps the residual out of denormals)

TRACE = bool(int(os.environ.get("BASS_KERNEL_TRACE", "0")))
LAST_EXEC_NS = None
LAST_RESULTS = None

_NC_CACHE = {}


def _build_bass():
    if "nc" in _NC_CACHE:
        return _NC_CACHE["nc"]

    import concourse.bacc as bacc
    import concourse.tile as tile
    import concourse.mybir as mybir
    from concourse.masks import make_identity

    fp32 = mybir.dt.float32
    bf16 = mybir.dt.bfloat16
    f8 = mybir.dt.float8e4
    DRmode = mybir.MatmulPerfMode.DoubleRow
    Act = mybir.ActivationFunctionType

    nc = bacc.Bacc("TRN2", target_bir_lowering=False)

    xt1 = nc.declare_dram_parameter("xt1", [D, L + 1], bf16, isOutput=False)
    # layer-1 contraction rows [0 : FP8U*128) as fp8e4m3 split units:
    # xt1f8 unit j = [hi(128 rows); lo(128 rows)], w1f8 unit j = [W*S8, W*S8/C8]
    # slot-paired for DoubleRow (0.5 cycles/row).
    xt1f8 = nc.declare_dram_parameter("xt1f8", [FP8U * 256, L + 1], f8,
                                      isOutput=False)
    w1f8 = nc.declare_dram_parameter("w1f8", [FP8U * 128, 2, D], f8,
                                     isOutput=False)
    xrb2 = nc.declare_dram_parameter("xrb2", [L, D], fp32, isOutput=False)
    # bf16 copy of the residual for the very last 128x128 output chunk: it is
    # injected into PSUM via an identity matmul so the final rstd chain skips
    # the DVE add (see phase B).
    xrl = nc.declare_dram_parameter("xrl", [128, 512], bf16, isOutput=False)
    c2 = nc.declare_dram_parameter("c2", [H, 1], bf16, isOutput=False)
    w1 = nc.declare_dram_parameter("w1", [D, D], bf16, isOutput=False)
    w2 = nc.declare_dram_parameter("w2", [H, 2 * D], bf16, isOutput=False)
    b1 = nc.declare_dram_parameter("b1", [H, 1], fp32, isOutput=False)
    out = nc.declare_dram_parameter("out", [L, D], fp32, isOutput=True)

    with tile.TileContext(nc) as tc, \
            tc.tile_pool(name="w1p", bufs=1) as w1p, \
            tc.tile_pool(name="w2p", bufs=1) as w2p, \
            tc.tile_pool(name="xt2p", bufs=1) as xt2p, \
            tc.tile_pool(name="x1p", bufs=1) as x1p, \
            tc.tile_pool(name="xrp", bufs=1) as xrp, \
            tc.tile_pool(name="rowp", bufs=2) as rowp, \
            tc.tile_pool(name="tmp", bufs=2) as tmp, \
            tc.tile_pool(name="const", bufs=1) as const, \
            tc.tile_pool(name="ps", bufs=1, space="PSUM") as psp:

        # PE clock warmup input: memset first so the dummy matmuls can start
        # as early as possible (the tensor engine ramps 0.65->1.2->2.4 GHz
        # with ~3us of sustained work; the first real matmul can't start
        # before its DMAs land at ~3.6us).
        n_warm = 10
        warm_free = 256
        dum = const.tile([128, BLK], bf16)
        nc.vector.memset(dum[:, 0:256], 0.0)

        # ------------- startup DMA stream (issue order == transfer order) ----
        # x1 tile (b0,k0) first, then W1 pair 0 in half-row pieces so the first
        # matmul unblocks after ~1.2 MB of DMA.
        K0 = FP8U                          # first bf16 k-tile (11)
        x1k = {}
        for k in range(K0, KT1):
            x1k[k] = x1p.tile([128, BLK + 1], bf16, tag=f"x1_{k}",
                              name=f"x1_0_{k}")
        x1f8 = []
        for j in range(FP8U):
            x1f8.append(x1p.tile([128, 2, BLK + 1], f8, tag=f"x1f8_{j}",
                                 name=f"x1f8_0_{j}"))
        w1s = {}
        for k in range(K0, KT1):
            w1s[k] = w1p.tile([128, D], bf16, tag=f"w1_{k}", name=f"w1_{k}")
        w1f8t = []
        for j in range(FP8U):
            w1f8t.append(w1p.tile([128, 2, D], f8, tag=f"w1f8_{j}",
                                  name=f"w1f8_{j}"))

        # The first real matmul is (k=K0, tap0, q0).  x1_{K0} goes through the
        # Pool/SWDGE path: its descriptor generation runs in parallel with the
        # HWDGE generation of the w1 pieces, so the first matmul's two
        # dependencies pipeline instead of serializing.
        r0 = K0 * 128
        nc.gpsimd.dma_start(out=x1k[K0], in_=xt1[r0:r0 + 128, 0:BLK + 1])
        for p in range(4):
            nc.sync.dma_start(out=w1s[K0][:, p * BLK:(p + 1) * BLK],
                              in_=w1[r0:r0 + 128, p * BLK:(p + 1) * BLK])
        nc.sync.dma_start(out=x1k[K0 + 1], in_=xt1[r0 + 128:r0 + 256, 0:BLK + 1])
        nc.sync.dma_start(out=w1s[K0 + 1][:, 0:H], in_=w1[r0 + 128:r0 + 256, 0:H])
        nc.sync.dma_start(out=w1s[K0 + 1][:, H:D], in_=w1[r0 + 128:r0 + 256, H:D])
        nc.sync.dma_start(out=x1k[K0 + 2], in_=xt1[r0 + 256:r0 + 384, 0:BLK + 1])
        for k in range(K0 + 2, KT1):
            nc.sync.dma_start(out=w1s[k][:, :], in_=w1[k * 128:(k + 1) * 128, :])
            if k + 1 < KT1:
                nc.sync.dma_start(
                    out=x1k[k + 1],
                    in_=xt1[(k + 1) * 128:(k + 2) * 128, 0:BLK + 1])
        # fp8 unit tiles (consumed at the END of each block -- plenty of slack)
        for j in range(FP8U):
            nc.sync.dma_start(
                out=x1f8[j],
                in_=xt1f8[j * 256:(j + 1) * 256, 0:BLK + 1].rearrange(
                    "(two p) c -> p two c", two=2))
            nc.sync.dma_start(out=w1f8t[j], in_=w1f8[j * 128:(j + 1) * 128, :, :])

        b1sb = const.tile([128, QT1, 1], fp32)
        epssb = const.tile([128, 1], fp32)
        xt2sb = xt2p.tile([128, KT2, L + 1], bf16)
        sqdump = const.tile([128, BLK], fp32)
        ident = const.tile([128, 128], bf16)
        xrlsb = const.tile([128, 512], bf16)
        nc.sync.dma_start(out=b1sb, in_=b1.rearrange("(q p) o -> p q o", p=128))
        nc.sync.dma_start(
            out=xt2sb[:, :, 0:1], in_=c2.rearrange("(k p) o -> p k o", p=128))
        nc.sync.dma_start(out=xrlsb, in_=xrl[:, :])
        nc.vector.memset(epssb, EPS)
        make_identity(nc, ident)

        # PE clock warmup: throwaway matmuls on the memset tile while the
        # first weight/activation DMAs are in flight, so the real matmuls
        # start at full clock.  Results go to a PSUM bank that the first real
        # accumulation group overwrites (start=True).
        if n_warm:
            wps = psp.tile([128, BLK], fp32, tag="ps0", name="ps_warm")
            for i in range(n_warm):
                nc.tensor.matmul(
                    wps[:, 0:warm_free], lhsT=dum[:, 0:128],
                    rhs=dum[:, 0:warm_free],
                    start=(i == 0), stop=(i == n_warm - 1))

        w2t = []
        for k in range(KT2):
            w2t.append(w2p.tile([128, 2 * D], bf16, tag=f"w2_{k}",
                                name=f"w2_{k}"))
        xrt = []
        for j in range(4):
            xrt.append(xrp.tile([128, BLK], fp32, tag=f"xr_{j}",
                                name=f"xr_0_{j}"))

        # ---------------- Phase A: layer 1 -> xt2sb (o1T, bf16) --------------
        for b in range(NBLK):
            psA = [psp.tile([128, BLK], fp32, tag=f"ps{q}", name=f"psA_{b}_{q}")
                   for q in range(QT1)]
            def bf16_k(k, start, stop, b=b):
                xk = x1k[k]
                for q in range(QT1):
                    nc.tensor.matmul(
                        psA[q], lhsT=w1s[k][:, q * 128:(q + 1) * 128],
                        rhs=xk[:, 0:BLK], start=start, stop=False)
                for q in range(QT1):
                    nc.tensor.matmul(
                        psA[q],
                        lhsT=w1s[k][:, H + q * 128:H + (q + 1) * 128],
                        rhs=xk[:, 1:BLK + 1], start=False, stop=stop)
                if b < NBLK - 1:
                    # refresh this k-slot for the next block (WAR dep on the
                    # 16 matmuls just issued -- already satisfied when the DMA
                    # reaches the head of the queue).
                    x1k[k] = x1p.tile([128, BLK + 1], bf16, tag=f"x1_{k}",
                                      name=f"x1_{b + 1}_{k}")
                    nc.sync.dma_start(
                        out=x1k[k],
                        in_=xt1[k * 128:(k + 1) * 128,
                                (b + 1) * BLK:(b + 1) * BLK + BLK + 1])

            for k in range(K0, KT1 - 1):
                bf16_k(k, start=(k == K0), stop=False)
            # fp8 DoubleRow pairs (k-tiles 0..2*FP8P-1): two k-tiles per pass
            # at 0.5 cycles/row -- 4x fewer PE cycles than the bf16 pairs
            # they replace.  k=KT1-1 (bf16) runs AFTER them so the slow taps
            # hide the PSUM drain ladder at the block boundary.
            for j in range(FP8U):
                xp = x1f8[j]
                for q in range(QT1):
                    nc.tensor.matmul(
                        psA[q], lhsT=w1f8t[j][:, :, q * 128:(q + 1) * 128],
                        rhs=xp[:, :, 0:BLK], start=False, stop=False,
                        perf_mode=DRmode)
                for q in range(QT1):
                    nc.tensor.matmul(
                        psA[q],
                        lhsT=w1f8t[j][:, :, H + q * 128:H + (q + 1) * 128],
                        rhs=xp[:, :, 1:BLK + 1], start=False, stop=False,
                        perf_mode=DRmode)
                if b < NBLK - 1:
                    x1f8[j] = x1p.tile([128, 2, BLK + 1], f8, tag=f"x1f8_{j}",
                                       name=f"x1f8_{b + 1}_{j}")
                    nc.sync.dma_start(
                        out=x1f8[j],
                        in_=xt1f8[j * 256:(j + 1) * 256,
                                  (b + 1) * BLK:(b + 1) * BLK + BLK + 1]
                        .rearrange("(two p) c -> p two c", two=2))
            bf16_k(KT1 - 1, start=False, stop=True)
            # drain PSUM -> xt2sb on two engines so the next block's first
            # matmuls (WAR on these banks) aren't gated by one engine's
            # serial drain ladder.
            for q in range(QT1):
                if q % 2 == 0:
                    nc.scalar.activation(
                        out=xt2sb[:, q, 1 + b * BLK:1 + (b + 1) * BLK],
                        in_=psA[q],
                        func=Act.Identity, bias=b1sb[:, q, :], scale=1.0)
                else:
                    nc.vector.tensor_scalar_add(
                        out=xt2sb[:, q, 1 + b * BLK:1 + (b + 1) * BLK],
                        in0=psA[q], scalar1=b1sb[:, q, :])
            # stagger W2 loads across blocks 0..2 so they never gate phase B
            for k in {0: (0, 1, 2), 1: (3, 4, 5), 2: (6, 7)}.get(b, ()):
                nc.sync.dma_start(out=w2t[k], in_=w2[k * 128:(k + 1) * 128, :])
            if b == NBLK - 1:
                # first token-tile's residual chunks for phase B
                for j in range(4):
                    nc.sync.dma_start(
                        out=xrt[j], in_=xrb2[0:128, j * BLK:(j + 1) * BLK])

        # ---------- Phase B: layer 2 token-major + residual + RMSNorm --------
        for t in range(TT):
            t0 = t * 128
            last = t == TT - 1
            rowc = rowp.tile([128, D], fp32, tag="row", name=f"row_{t}")
            # final tile: the last feature chunk is only 128 wide so the
            # add/square on the rstd critical path after the very last matmul
            # is short.
            chunks = [(0, BLK), (BLK, BLK), (2 * BLK, BLK),
                      (3 * BLK, 384), (3 * BLK + 384, 128)] if last else \
                     [(0, BLK), (BLK, BLK), (2 * BLK, BLK), (3 * BLK, BLK)]
            acc = tmp.tile([128, 4], fp32, tag="acc", name=f"acc_{t}")
            acc2 = tmp.tile([128, 1], fp32, tag="acc2", name=f"acc2_{t}") \
                if last else None
            ps_final = {}
            for ci, (c0, cw) in enumerate(chunks):
                f4 = c0 // BLK
                final = last and ci == 3
                ps = psp.tile([128, BLK], fp32, tag=f"ps{(t * 4 + ci) % 8}",
                              name=f"psB_{t}_{ci}")
                for k in range(KT2):
                    nc.tensor.matmul(
                        ps[:, 0:cw], lhsT=xt2sb[:, k, t0:t0 + 128],
                        rhs=w2t[k][:, c0:c0 + cw],
                        start=(k == 0), stop=False)
                for k in range(KT2):
                    nc.tensor.matmul(
                        ps[:, 0:cw], lhsT=xt2sb[:, k, t0 + 1:t0 + 129],
                        rhs=w2t[k][:, D + c0:D + c0 + cw],
                        start=False, stop=(k == KT2 - 1 and not final))
                sl = slice(c0, c0 + cw)
                if final:
                    # inject the residual into PSUM with an identity matmul;
                    # square directly from PSUM on Act (one PSUM input is
                    # legal, two are not -- NCC_IBVF027), freeing the DVE for
                    # the very last chunk's add+square chain.
                    x0 = c0 - 3 * BLK
                    nc.tensor.matmul(
                        ps[:, 0:cw], lhsT=ident, rhs=xrlsb[:, x0:x0 + cw],
                        start=False, stop=True)
                    nc.scalar.activation(
                        out=sqdump[:, 0:cw], in_=ps[:, 0:cw],
                        func=Act.Square, accum_out=acc[:, ci:ci + 1])
                    ps_final[ci] = ps
                    continue
                if last and ci == 4:
                    # last chunk: add -> SBUF on DVE (one PSUM input), then
                    # sum-of-squares on DVE into a separate accumulator so it
                    # is not serialized behind Act's chunk-3a square (WAW on
                    # acc).
                    nc.vector.tensor_add(
                        out=rowc[:, sl], in0=ps[:, 0:cw],
                        in1=xrt[f4][:, c0 - f4 * BLK:c0 - f4 * BLK + cw])
                    nc.vector.affine_mul_reduce(
                        out=sqdump[:, 0:cw], accum_out=acc2,
                        in0=rowc[:, sl], in1=rowc[:, sl], scale=1.0, bias=0.0)
                    continue
                nc.vector.tensor_add(out=rowc[:, sl], in0=ps[:, 0:cw],
                                     in1=xrt[f4][:, c0 - f4 * BLK:c0 - f4 * BLK + cw])
                if not last and ci < 4:
                    xrt[ci] = xrp.tile([128, BLK], fp32, tag=f"xr_{ci}",
                                       name=f"xr_{t + 1}_{ci}")
                    nc.sync.dma_start(
                        out=xrt[ci],
                        in_=xrb2[t0 + 128:t0 + 256, ci * BLK:(ci + 1) * BLK])
                nc.scalar.activation(
                    out=sqdump[:, 0:cw], in_=rowc[:, sl],
                    func=Act.Square, accum_out=acc[:, ci:ci + 1])
            rstd = tmp.tile([128, 1], fp32, tag="rstd", name=f"rstd_{t}")
            nc.vector.tensor_reduce(
                out=rstd, in_=acc, axis=mybir.AxisListType.X,
                op=mybir.AluOpType.add)
            if last:
                nc.vector.tensor_add(out=rstd, in0=rstd, in1=acc2)
            nc.scalar.activation(
                out=rstd, in_=rstd, func=Act.Sqrt, bias=epssb, scale=1.0 / D)
            nc.vector.reciprocal(out=rstd, in_=rstd)
            if last:
                # 4-way scale/store, DVE first, so the first store's
                # descriptors enter the (serialized) DMA path asap.  The last
                # 128 columns are scaled straight out of PSUM (ps_final).
                nc.vector.tensor_scalar_mul(
                    out=rowc[:, 0:BLK], in0=rowc[:, 0:BLK], scalar1=rstd)
                nc.sync.dma_start(out=out[t0:t0 + 128, 0:BLK],
                                  in_=rowc[:, 0:BLK])
                nc.scalar.activation(
                    out=rowc[:, BLK:2 * BLK], in_=rowc[:, BLK:2 * BLK],
                    func=Act.Identity, bias=0.0, scale=rstd)
                nc.sync.dma_start(out=out[t0:t0 + 128, BLK:2 * BLK],
                                  in_=rowc[:, BLK:2 * BLK])
                nc.vector.tensor_scalar_mul(
                    out=rowc[:, 2 * BLK:3 * BLK], in0=rowc[:, 2 * BLK:3 * BLK],
                    scalar1=rstd)
                nc.sync.dma_start(out=out[t0:t0 + 128, 2 * BLK:3 * BLK],
                                  in_=rowc[:, 2 * BLK:3 * BLK])
                nc.scalar.activation(
                    out=rowc[:, 3 * BLK:3 * BLK + 384],
                    in_=ps_final[3][:, 0:384],
                    func=Act.Identity, bias=0.0, scale=rstd)
                nc.vector.tensor_scalar_mul(
                    out=rowc[:, 3 * BLK + 384:D], in0=rowc[:, 3 * BLK + 384:D],
                    scalar1=rstd)
                nc.sync.dma_start(out=out[t0:t0 + 128, 3 * BLK:D],
                                  in_=rowc[:, 3 * BLK:D])
            else:
                nc.scalar.activation(
                    out=rowc[:, 0:H], in_=rowc[:, 0:H],
                    func=Act.Identity, bias=0.0, scale=rstd)
                nc.vector.tensor_scalar_mul(
                    out=rowc[:, H:D], in0=rowc[:, H:D], scalar1=rstd)
                nc.sync.dma_start(out=out[t0:t0 + 128, 0:H], in_=rowc[:, 0:H])
                nc.sync.dma_start(out=out[t0:t0 + 128, H:D], in_=rowc[:, H:D])

    nc.finalize()
    _NC_CACHE["nc"] = nc
    return nc


def _np_reference(inputs, pre_lf_indexs, out_lf_indexs, input_lf_loc, out_lf_loc,
                  inputs_loc, outputs_loc, lf1_caches, lf2_caches,
                  conv1_weight, conv2_weight, conv1_bias, conv2_bias, ln_weight):
    """Generic numpy fallback (only used if the index structure is unexpected)."""
    def fused(x, cache, pre_idx, in_lf_loc, in_loc, out_loc, W):
        bs = pre_idx.shape[0]
        xt = np.zeros((x.shape[0] + bs, x.shape[1]), x.dtype)
        xt[in_loc] = x
        xt[in_lf_loc] = cache[pre_idx]
        c = xt @ W
        h = c.shape[1] // 2
        y = c[:-1, :h] + c[1:, h:]
        return y[out_loc]

    o1 = fused(inputs, lf1_caches, pre_lf_indexs, input_lf_loc,
               inputs_loc, outputs_loc, conv1_weight) + conv1_bias
    o2 = fused(o1, lf2_caches, pre_lf_indexs, input_lf_loc,
               inputs_loc, outputs_loc, conv2_weight) + conv2_bias
    o3 = o2 + inputs
    var = np.mean(o3 * o3, axis=-1, keepdims=True)
    return (o3 / np.sqrt(var + EPS) * ln_weight).astype(np.float32)


def kernel(**inputs):
    global LAST_EXEC_NS, LAST_RESULTS
    inp = {k: np.asarray(v) for k, v in inputs.items()}
    x = inp["inputs"].astype(np.float32, copy=False)
    lnw = inp["ln_weight"].astype(np.float32, copy=False)

    s = np.arange(BS, dtype=np.int64)
    j = np.arange(L, dtype=np.int64)
    structured = (
        np.array_equal(inp["inputs_loc"], (s[:, None] * (L + 1) + 1 + j[None, :]).reshape(-1))
        and np.array_equal(inp["outputs_loc"], (s[:, None] * (L + 1) + j[None, :]).reshape(-1))
        and np.array_equal(inp["input_lf_loc"], s * (L + 1))
    )
    if not structured:
        return _np_reference(**inp)

    from concourse.bass_utils import run_bass_kernel_spmd

    nc = _build_bass()

    bf16 = ml_dtypes.bfloat16
    e4 = ml_dtypes.float8_e4m3
    pre_idx = inp["pre_lf_indexs"].astype(np.int64)
    w1f = inp["conv1_weight"].astype(np.float32)
    w1b = np.ascontiguousarray(w1f.astype(bf16))
    # W1 rows [0 : FP8U*128) as DoubleRow split units [unit*128, 2, D]:
    # slot0 = q(W*S8), slot1 = q(W*S8/C8) (lo-residual compensation slot)
    wa = (w1f[:FP8U * 128] * S8).astype(e4)
    wb = (w1f[:FP8U * 128] * (S8 / C8)).astype(e4)
    w1f8 = np.ascontiguousarray(
        np.stack([wa.reshape(FP8U, 128, D), wb.reshape(FP8U, 128, D)],
                 axis=2).reshape(FP8U * 128, 2, D))
    w2b = np.ascontiguousarray(inp["conv2_weight"].astype(bf16))
    b1f = np.ascontiguousarray(inp["conv1_bias"].astype(np.float32).reshape(H, 1))
    b2row = inp["conv2_bias"].astype(np.float32).reshape(1, D)

    def xt1f8_of(a):
        xa = a[:FP8U * 128] * np.float32(1.0 / S8)
        hi = xa.astype(e4)
        lo = ((xa - hi.astype(np.float32)) * np.float32(C8)).astype(e4)
        return np.ascontiguousarray(
            np.stack([hi.reshape(FP8U, 128, L + 1),
                      lo.reshape(FP8U, 128, L + 1)], axis=1)
            .reshape(FP8U * 256, L + 1))

    in_maps = []
    for sq in range(BS):
        xs = x[sq * L:(sq + 1) * L]                       # [2048, 2048]
        a = np.empty((D, L + 1), np.float32)
        a[:, 0] = inp["lf1_caches"][pre_idx[sq]]
        a[:, 1:] = xs.T
        xrb2 = xs + b2row
        in_maps.append({
            "xt1": np.ascontiguousarray(a.astype(bf16)),
            "xt1f8": xt1f8_of(a),
            "w1f8": w1f8,
            "xrb2": np.ascontiguousarray(xrb2),
            "xrl": np.ascontiguousarray(
                xrb2[L - 128:, D - 512:].astype(bf16)),
            "c2": np.ascontiguousarray(
                inp["lf2_caches"][pre_idx[sq]].astype(bf16).reshape(H, 1)),
            "w1": w1b,
            "w2": w2b,
            "b1": b1f,
        })

    res = run_bass_kernel_spmd(nc, in_maps, list(range(NCORES)), trace=TRACE)
    LAST_EXEC_NS = res.exec_time_ns
    LAST_RESULTS = res
    out = np.concatenate([res.results[i]["out"] for i in range(NCORES)], axis=0)
    if not np.all(lnw == 1.0):
        out = out * lnw[None, :]
    return out.astype(np.float32)
